# revision 18
# baseline (speedup 1.0000x reference)
"""Bidirectional cross-attention Trainium2 kernel.

Data-parallel over batch B=8 across 8 NeuronCores (1 sample/core).

Per-core dataflow (matmul operands fp16, attention weights bf16, fp32 accum):
  Q1[c,n], K2[c,n]   : 1x1-conv projections (fp16), per-partition bias on ACT/DVE
  V2T[n,c]+bias (bf16), plus ones columns at c=256/257 (bias folded via K=1 matmul)
  expS1T[j,i] = exp(K2^T Q1)  (ScalarE exp -> bf16; no max-subtract: logits are
                               bounded ~|41| for this problem's scale, exp fits fp32/bf16)
  outT[i, 0:258] = sum_j expS1T[j,i] * [V2T+b | 1 | 1][j, :]
     -> col 256 is the softmax denominator; y = outT[:,0:256]/denominator + x1T
  (symmetric for direction 2)
Chunks of 512 i-columns are software-pipelined: expS(k) matmuls+exps woven with
out(k-1) matmuls so PE never waits on ScalarE; dummy warm-up matmuls keep the
PE HAM clock at 2.4GHz during the input DMA window.

Host side: shard batch over cores, transpose weights/x, gather + transpose outputs.
"""

import sys

if "/opt/trn_rl_repo" not in sys.path:
    sys.path.insert(0, "/opt/trn_rl_repo")

import numpy as np

B, C, H, W = 8, 256, 48, 48
N = H * W  # 2304
NT = N // 128  # 18 j/i tiles
CT = C // 128  # 2 c tiles
CW = 512  # max i-chunk width for expS (last chunk is 256)
CHUNKS = [(0, 512), (512, 512), (1024, 512), (1536, 512), (2048, 256)]

_CACHE = {}


def _build():
    import concourse.bacc as bacc
    import concourse.mybir as mybir
    from concourse.tile import TileContext

    F32, F16, BF16 = mybir.dt.float32, mybir.dt.float16, mybir.dt.bfloat16
    Exp = mybir.ActivationFunctionType.Exp
    Ident = mybir.ActivationFunctionType.Identity

    nc = bacc.Bacc(None, target_bir_lowering=False)

    x_d = {
        "x1": nc.dram_tensor("x1", [C, N], F16, kind="ExternalInput"),
        "x2": nc.dram_tensor("x2", [C, N], F16, kind="ExternalInput"),
    }
    xt_d = {
        "x1t": nc.dram_tensor("x1t", [N, C], F32, kind="ExternalInput"),
        "x2t": nc.dram_tensor("x2t", [N, C], F32, kind="ExternalInput"),
    }
    w_names = ["wq1t", "wk2t", "wv2t", "wq2t", "wk1t", "wv1t"]  # pack order
    wpack_d = nc.dram_tensor("wpack", [C, 6 * C], F16, kind="ExternalInput")
    bqk_names = ["bq1", "bk1", "bq2", "bk2"]  # pack order
    bqk_d = nc.dram_tensor("bqk", [C, 4], F32, kind="ExternalInput")
    # bvpack: [bv1(258) | bv2(258) | ones(128)]
    bvpack_d = nc.dram_tensor("bvpack", [1, 644], F16, kind="ExternalInput")
    y_d = {
        "y1t": nc.dram_tensor("y1t", [N, C], F32, kind="ExternalOutput"),
        "y2t": nc.dram_tensor("y2t", [N, C], F32, kind="ExternalOutput"),
    }

    with TileContext(nc) as tc:
        with (
            tc.tile_pool(name="const", bufs=1) as cp,
            tc.tile_pool(name="proj", bufs=1) as pp,
            tc.tile_pool(name="stream", bufs=4) as sp,
            tc.tile_pool(name="psum", bufs=2, space="PSUM") as psp,
            tc.tile_pool(name="psum_s", bufs=3, space="PSUM") as psp2,
        ):
            # ---------- projections (x tiles freed after this block) ----------
            proj = {}
            with tc.tile_pool(name="xp", bufs=1) as xp:
                # x1 first (feeds the first projection), then packed consts, then x2
                x_sb = {}

                def load_x(n):
                    t = xp.tile([128, CT, N], F16, tag=n)
                    for ck in range(CT):
                        for h0 in (0, N // 2):
                            nc.sync.dma_start(
                                out=t[:, ck, h0 : h0 + N // 2],
                                in_=x_d[n][ck * 128 : (ck + 1) * 128, h0 : h0 + N // 2],
                            )
                    x_sb[n] = t

                # PE warm-up while input DMAs are in flight: keeps HAM at 8/8
                dummy = cp.tile([128, 512], F16, tag="warm")
                nc.vector.memset(dummy[:, :], 0.0)
                wps = psp.tile([128, 512], F32, tag="ps_o")
                for _ in range(64):
                    nc.tensor.matmul(
                        wps[:, :], dummy[:, 0:128], dummy[:, :], start=True, stop=True
                    )
                wexp = cp.tile([128, 512], F32, tag="warm_exp")
                nc.scalar.activation(wexp[:, :], wps[:, :], Exp)

                load_x("x1")
                wpack = cp.tile([128, CT, 6 * C], F16, tag="wpack")
                for ck in range(CT):
                    nc.sync.dma_start(
                        out=wpack[:, ck, :], in_=wpack_d[ck * 128 : (ck + 1) * 128, :]
                    )
                w_sb = {
                    n: wpack[:, :, i * C : (i + 1) * C] for i, n in enumerate(w_names)
                }
                bqkt = cp.tile([128, CT, 4], F32, tag="bqkt")
                for ck in range(CT):
                    nc.sync.dma_start(
                        out=bqkt[:, ck, :], in_=bqk_d[ck * 128 : (ck + 1) * 128, :]
                    )
                bqk_sb = {n: bqkt[:, :, i] for i, n in enumerate(bqk_names)}
                bvpack = cp.tile([1, 644], F16, tag="bvpack")
                nc.sync.dma_start(out=bvpack[:, :], in_=bvpack_d[:, :])
                bv_sb = {"bv1": bvpack[:, 0:258], "bv2": bvpack[:, 258:516]}
                ones_sb = bvpack[:, 516:644]
                load_x("x2")

                def proj_qk(dst, xt, wn, bn):
                    alt = 0
                    for ct in range(CT):
                        for c0, cw in CHUNKS:
                            ps2 = psp2.tile([128, 2, CW], F32, tag="ps_s")
                            ps = ps2[:, 0, :]
                            for ck in range(CT):
                                nc.tensor.matmul(
                                    ps[:, 0:cw],
                                    w_sb[wn][:, ck, ct * 128 : (ct + 1) * 128],
                                    xt[:, ck, c0 : c0 + cw],
                                    start=(ck == 0),
                                    stop=(ck == CT - 1),
                                )
                            if alt % 2 == 0:
                                nc.scalar.activation(
                                    dst[:, ct, c0 : c0 + cw],
                                    ps[:, 0:cw],
                                    Ident,
                                    bias=bqk_sb[bn][:, ct : ct + 1],
                                )
                            else:
                                nc.vector.tensor_scalar_add(
                                    dst[:, ct, c0 : c0 + cw],
                                    ps[:, 0:cw],
                                    bqk_sb[bn][:, ct : ct + 1],
                                )
                            alt += 1

                def proj_vt(dst, xt, wn, bn):
                    for jt in range(NT):
                        ps2 = psp2.tile([128, 2, CW], F32, tag="ps_s")
                        ps = ps2[:, 0, :]
                        for ck in range(CT):
                            nc.tensor.matmul(
                                ps[:, 0:C],
                                xt[:, ck, jt * 128 : (jt + 1) * 128],
                                w_sb[wn][:, ck, :],
                                start=(ck == 0),
                                stop=False,
                                skip_group_check=True,
                            )
                        nc.tensor.matmul(
                            ps[:, 0 : C + 2],
                            ones_sb,
                            bv_sb[bn],
                            start=False,
                            stop=True,
                            skip_group_check=True,
                        )
                        if jt % 2 == 0:
                            nc.vector.tensor_copy(dst[:, jt, :], ps[:, 0 : C + 2])
                        else:
                            nc.scalar.activation(dst[:, jt, :], ps[:, 0 : C + 2], Ident)

                for nm, xn, wn, bn in [
                    ("Q1", "x1", "wq1t", "bq1"),
                    ("K2", "x2", "wk2t", "bk2"),
                    ("Q2", "x2", "wq2t", "bq2"),
                    ("K1", "x1", "wk1t", "bk1"),
                ]:
                    t = pp.tile([128, CT, N], F16, tag=nm)
                    proj_qk(t, x_sb[xn], wn, bn)
                    proj[nm] = t
                for nm, xn, wn, bn in [
                    ("VT2", "x2", "wv2t", "bv2"),
                    ("VT1", "x1", "wv1t", "bv1"),
                ]:
                    t = pp.tile([128, NT, C + 2], BF16, tag=nm)
                    proj_vt(t, x_sb[xn], wn, bn)
                    proj[nm] = t

            # ---------- attention, one direction at a time ----------
            with tc.tile_pool(name="ep", bufs=2) as ep:

                def exp_actions(Q, K, e, c0, cw):
                    # one action = expS matmuls + one wide exp for a PAIR of j-tiles
                    def mk(jp):
                        def act():
                            ps2 = psp2.tile([128, 2, CW], F32, tag="ps_s")
                            for jj in range(2):
                                jt = jp + jj
                                for ck in range(CT):
                                    nc.tensor.matmul(
                                        ps2[:, jj, 0:cw],
                                        K[:, ck, jt * 128 : (jt + 1) * 128],
                                        Q[:, ck, c0 : c0 + cw],
                                        start=(ck == 0),
                                        stop=(ck == CT - 1),
                                    )
                            nc.scalar.activation(
                                e[:, jp : jp + 2, 0:cw], ps2[:, :, 0:cw], Exp
                            )

                        return act

                    return [mk(jp) for jp in range(0, NT, 2)]

                def out_actions(e, VT, xt_dram, yt_dram, c0, cw):
                    # actions = out-matmul slices + epilogue, per i-subtile
                    acts = []
                    for il in range(cw // 128):
                        it = c0 // 128 + il
                        po = psp.tile([128, C + 2], F32, tag="ps_o")

                        xt_t = sp.tile([128, C], F32, tag="xt")

                        def mk_mm(po, il, it, j0, jn, xt_t):
                            def act():
                                if j0 == 0:
                                    nc.sync.dma_start(
                                        out=xt_t[:, :],
                                        in_=xt_dram[it * 128 : (it + 1) * 128, :],
                                    )
                                for jt in range(j0, jn):
                                    nc.tensor.matmul(
                                        po[:, :],
                                        e[:, jt, il * 128 : (il + 1) * 128],
                                        VT[:, jt, :],
                                        start=(jt == 0),
                                        stop=(jt == NT - 1),
                                    )

                            return act

                        for j0 in range(0, NT, 5):
                            acts.append(mk_mm(po, il, it, j0, min(j0 + 5, NT), xt_t))

                        def mk_epi(po, it, xt_t):
                            def act():
                                r = sp.tile([128, 1], F32, tag="r")
                                nc.vector.reciprocal(r[:, :], po[:, C : C + 1])
                                y = sp.tile([128, C], F32, tag="y")
                                nc.vector.scalar_tensor_tensor(
                                    y[:, :],
                                    po[:, 0:C],
                                    r[:, :],
                                    xt_t[:, :],
                                    op0=mybir.AluOpType.mult,
                                    op1=mybir.AluOpType.add,
                                )
                                nc.sync.dma_start(
                                    out=yt_dram[it * 128 : (it + 1) * 128, :], in_=y[:, :]
                                )

                            return act

                        acts.append(mk_epi(po, it, xt_t))
                    return acts

                def weave(a, b):
                    # emit all of a and b interleaved evenly (a paces, b fills)
                    if not b:
                        for f in a:
                            f()
                        return
                    na, nb = len(a), len(b)
                    j = 0
                    for i, f in enumerate(a):
                        f()
                        while j < nb and j * na <= (i + 1) * nb - 1:
                            b[j]()
                            j += 1
                    while j < nb:
                        b[j]()
                        j += 1

                # software pipeline across both directions: expS(k) woven with out(k-1)
                plan = [
                    (proj["Q1"], proj["K2"], proj["VT2"], xt_d["x1t"], y_d["y1t"], c0, cw)
                    for c0, cw in CHUNKS
                ] + [
                    (proj["Q2"], proj["K1"], proj["VT1"], xt_d["x2t"], y_d["y2t"], c0, cw)
                    for c0, cw in CHUNKS
                ]
                pending = []
                for Q, K, VT, xtd, ytd, c0, cw in plan:
                    e = ep.tile([128, NT, CW], BF16, tag="e")
                    weave(exp_actions(Q, K, e, c0, cw), pending)
                    pending = out_actions(e, VT, xtd, ytd, c0, cw)
                weave(pending, [])

    nc.compile()
    return nc


def _get_nc():
    if "nc" not in _CACHE:
        _CACHE["nc"] = _build()
    return _CACHE["nc"]


def kernel(
    x1,
    x2,
    w_q1,
    b_q1,
    w_k1,
    b_k1,
    w_v1,
    b_v1,
    w_q2,
    b_q2,
    w_k2,
    b_k2,
    w_v2,
    b_v2,
    _trace=False,
):
    from concourse.bass_utils import run_bass_kernel_spmd

    nc = _get_nc()

    x1 = np.asarray(x1, dtype=np.float32)
    x2 = np.asarray(x2, dtype=np.float32)
    x1h = x1.astype(np.float16)
    x2h = x2.astype(np.float16)
    # wpack order must match w_names: wq1t, wk2t, wv2t, wq2t, wk1t, wv1t
    wpack = np.ascontiguousarray(
        np.concatenate(
            [np.asarray(w, np.float32).T for w in [w_q1, w_k2, w_v2, w_q2, w_k1, w_v1]],
            axis=1,
        ).astype(np.float16)
    )
    bqk = np.ascontiguousarray(
        np.stack(
            [np.asarray(b, np.float32) for b in [b_q1, b_k1, b_q2, b_k2]], axis=1
        )
    )
    bv1 = np.concatenate(
        [np.asarray(b_v1, np.float32).reshape(1, C), np.ones((1, 2), np.float32)], 1
    )
    bv2 = np.concatenate(
        [np.asarray(b_v2, np.float32).reshape(1, C), np.ones((1, 2), np.float32)], 1
    )
    bvpack = np.concatenate([bv1, bv2, np.ones((1, 128), np.float32)], 1).astype(
        np.float16
    )

    in_maps = []
    for i in range(B):
        x1i = np.ascontiguousarray(x1[i].reshape(C, N))
        x2i = np.ascontiguousarray(x2[i].reshape(C, N))
        m = {
            "x1": np.ascontiguousarray(x1h[i].reshape(C, N)),
            "x2": np.ascontiguousarray(x2h[i].reshape(C, N)),
            "x1t": np.ascontiguousarray(x1i.T),
            "x2t": np.ascontiguousarray(x2i.T),
            "wpack": wpack,
            "bqk": bqk,
            "bvpack": bvpack,
        }
        in_maps.append(m)

    res = run_bass_kernel_spmd(nc, in_maps, list(range(B)), trace=_trace)
    if _trace:
        _CACHE["last_result"] = res

    y1 = np.empty((B, C, H, W), np.float32)
    y2 = np.empty((B, C, H, W), np.float32)
    for i in range(B):
        y1[i] = res.results[i]["y1t"].T.reshape(C, H, W)
        y2[i] = res.results[i]["y2t"].T.reshape(C, H, W)
    return y1, y2


# revision 20
# speedup vs baseline: 1.0071x; 1.0071x over previous
"""Bidirectional cross-attention Trainium2 kernel.

Data-parallel over batch B=8 across 8 NeuronCores (1 sample/core).

Per-core dataflow (matmul operands fp16, attention weights bf16, fp32 accum):
  Q1[c,n], K2[c,n]   : 1x1-conv projections (fp16), per-partition bias on ACT/DVE
  V2T[n,c]+bias (bf16), plus ones columns at c=256/257 (bias folded via K=1 matmul)
  expS1T[j,i] = exp(K2^T Q1)  (ScalarE exp -> bf16; no max-subtract: logits are
                               bounded ~|41| for this problem's scale, exp fits fp32/bf16)
  outT[i, 0:258] = sum_j expS1T[j,i] * [V2T+b | 1 | 1][j, :]
     -> col 256 is the softmax denominator; y = outT[:,0:256]/denominator + x1T
  (symmetric for direction 2)
Chunks of 512 i-columns are software-pipelined: expS(k) matmuls+exps woven with
out(k-1) matmuls so PE never waits on ScalarE; dummy warm-up matmuls keep the
PE HAM clock at 2.4GHz during the input DMA window.

Host side: shard batch over cores, transpose weights/x, gather + transpose outputs.
"""

import sys

if "/opt/trn_rl_repo" not in sys.path:
    sys.path.insert(0, "/opt/trn_rl_repo")

import numpy as np

B, C, H, W = 8, 256, 48, 48
N = H * W  # 2304
NT = N // 128  # 18 j/i tiles
CT = C // 128  # 2 c tiles
CW = 512  # max i-chunk width for expS (last chunk is 256)
CHUNKS = [(0, 512), (512, 512), (1024, 512), (1536, 512), (2048, 256)]

_CACHE = {}


def _build():
    import concourse.bacc as bacc
    import concourse.mybir as mybir
    from concourse.tile import TileContext

    F32, F16, BF16 = mybir.dt.float32, mybir.dt.float16, mybir.dt.bfloat16
    Exp = mybir.ActivationFunctionType.Exp
    Ident = mybir.ActivationFunctionType.Identity

    nc = bacc.Bacc(None, target_bir_lowering=False)

    x_d = {
        "x1": nc.dram_tensor("x1", [C, N], F16, kind="ExternalInput"),
        "x2": nc.dram_tensor("x2", [C, N], F16, kind="ExternalInput"),
    }
    xt_d = {
        "x1t": nc.dram_tensor("x1t", [N, C], F32, kind="ExternalInput"),
        "x2t": nc.dram_tensor("x2t", [N, C], F32, kind="ExternalInput"),
    }
    w_names = ["wq1t", "wk2t", "wv2t", "wq2t", "wk1t", "wv1t"]  # pack order
    wpack_d = nc.dram_tensor("wpack", [C, 6 * C], F16, kind="ExternalInput")
    bqk_names = ["bq1", "bk1", "bq2", "bk2"]  # pack order
    bqk_d = nc.dram_tensor("bqk", [C, 4], F32, kind="ExternalInput")
    # bvpack: [bv1(258) | bv2(258) | ones(128)]
    bvpack_d = nc.dram_tensor("bvpack", [1, 644], F16, kind="ExternalInput")
    y_d = {
        "y1t": nc.dram_tensor("y1t", [N, C], F32, kind="ExternalOutput"),
        "y2t": nc.dram_tensor("y2t", [N, C], F32, kind="ExternalOutput"),
    }

    with TileContext(nc) as tc:
        with (
            tc.tile_pool(name="const", bufs=1) as cp,
            tc.tile_pool(name="proj", bufs=1) as pp,
            tc.tile_pool(name="stream", bufs=4) as sp,
            tc.tile_pool(name="psum", bufs=2, space="PSUM") as psp,
            tc.tile_pool(name="psum_s", bufs=3, space="PSUM") as psp2,
        ):
            # ---------- setup: warm-up + input loads ----------
            proj = {}
            # PE warm-up while input DMAs are in flight: keeps HAM at 8/8
            dummy = cp.tile([128, 512], F16, tag="warm")
            nc.vector.memset(dummy[:, :], 0.0)
            wps = psp.tile([128, 512], F32, tag="ps_o")
            for _ in range(64):
                nc.tensor.matmul(
                    wps[:, :], dummy[:, 0:128], dummy[:, :], start=True, stop=True
                )
            wexp = cp.tile([128, 512], F32, tag="warm_exp")
            nc.scalar.activation(wexp[:, :], wps[:, :], Exp)

            x_sb = {}

            def load_x(n):
                t = pp.tile([128, CT, N], F16, tag=n)
                for ck in range(CT):
                    for h0 in (0, N // 2):
                        nc.sync.dma_start(
                            out=t[:, ck, h0 : h0 + N // 2],
                            in_=x_d[n][ck * 128 : (ck + 1) * 128, h0 : h0 + N // 2],
                        )
                x_sb[n] = t

            load_x("x1")
            wpack = cp.tile([128, CT, 6 * C], F16, tag="wpack")
            for ck in range(CT):
                nc.sync.dma_start(
                    out=wpack[:, ck, :], in_=wpack_d[ck * 128 : (ck + 1) * 128, :]
                )
            w_sb = {n: wpack[:, :, i * C : (i + 1) * C] for i, n in enumerate(w_names)}
            bqkt = cp.tile([128, CT, 4], F32, tag="bqkt")
            for ck in range(CT):
                nc.sync.dma_start(
                    out=bqkt[:, ck, :], in_=bqk_d[ck * 128 : (ck + 1) * 128, :]
                )
            bqk_sb = {n: bqkt[:, :, i] for i, n in enumerate(bqk_names)}
            bvpack = cp.tile([1, 644], F16, tag="bvpack")
            nc.sync.dma_start(out=bvpack[:, :], in_=bvpack_d[:, :])
            bv_sb = {"bv1": bvpack[:, 0:258], "bv2": bvpack[:, 258:516]}
            ones_sb = bvpack[:, 516:644]
            load_x("x2")

            # ---------- projection action builders ----------
            def proj_qk_actions(dst, xt, wn, bn, alt0=0):
                acts = []
                i = 0
                for ct in range(CT):
                    for c0, cw in CHUNKS:

                        def mk(ct, c0, cw, use_act):
                            def act():
                                ps2 = psp2.tile([128, 2, CW], F32, tag="ps_s")
                                ps = ps2[:, 0, :]
                                for ck in range(CT):
                                    nc.tensor.matmul(
                                        ps[:, 0:cw],
                                        w_sb[wn][:, ck, ct * 128 : (ct + 1) * 128],
                                        xt[:, ck, c0 : c0 + cw],
                                        start=(ck == 0),
                                        stop=(ck == CT - 1),
                                    )
                                if use_act:
                                    nc.scalar.activation(
                                        dst[:, ct, c0 : c0 + cw],
                                        ps[:, 0:cw],
                                        Ident,
                                        bias=bqk_sb[bn][:, ct : ct + 1],
                                    )
                                else:
                                    nc.vector.tensor_scalar_add(
                                        dst[:, ct, c0 : c0 + cw],
                                        ps[:, 0:cw],
                                        bqk_sb[bn][:, ct : ct + 1],
                                    )

                            return act

                        acts.append(mk(ct, c0, cw, (alt0 + i) % 2 == 0))
                        i += 1
                return acts

            def proj_vt_actions(dst, xt, wn, bn):
                acts = []
                for jt in range(NT):

                    def mk(jt):
                        def act():
                            ps2 = psp2.tile([128, 2, CW], F32, tag="ps_s")
                            ps = ps2[:, 0, :]
                            for ck in range(CT):
                                nc.tensor.matmul(
                                    ps[:, 0:C],
                                    xt[:, ck, jt * 128 : (jt + 1) * 128],
                                    w_sb[wn][:, ck, :],
                                    start=(ck == 0),
                                    stop=False,
                                    skip_group_check=True,
                                )
                            nc.tensor.matmul(
                                ps[:, 0 : C + 2],
                                ones_sb,
                                bv_sb[bn],
                                start=False,
                                stop=True,
                                skip_group_check=True,
                            )
                            if jt % 2 == 0:
                                nc.vector.tensor_copy(dst[:, jt, :], ps[:, 0 : C + 2])
                            else:
                                nc.scalar.activation(
                                    dst[:, jt, :], ps[:, 0 : C + 2], Ident
                                )

                        return act

                    acts.append(mk(jt))
                return acts

            for nm in ["Q1", "K2", "Q2", "K1"]:
                proj[nm] = pp.tile([128, CT, N], F16, tag=nm, name=nm)
            for nm in ["VT2", "VT1"]:
                proj[nm] = pp.tile([128, NT, C + 2], BF16, tag=nm, name=nm)

            # direction-1 projections emitted up front
            for a in proj_qk_actions(proj["Q1"], x_sb["x1"], "wq1t", "bq1", 0):
                a()
            for a in proj_qk_actions(proj["K2"], x_sb["x2"], "wk2t", "bk2", 1):
                a()
            for a in proj_vt_actions(proj["VT2"], x_sb["x2"], "wv2t", "bv2"):
                a()
            # direction-2 projections become fill work woven into dir-1 attention
            fill = (
                proj_qk_actions(proj["Q2"], x_sb["x2"], "wq2t", "bq2", 0)
                + proj_qk_actions(proj["K1"], x_sb["x1"], "wk1t", "bk1", 1)
                + proj_vt_actions(proj["VT1"], x_sb["x1"], "wv1t", "bv1")
            )

            # ---------- attention ----------
            with tc.tile_pool(name="ep", bufs=2) as ep:

                def exp_actions(Q, K, e, c0, cw):
                    # one action = expS matmuls + one wide exp for a PAIR of j-tiles
                    def mk(jp):
                        def act():
                            ps2 = psp2.tile([128, 2, CW], F32, tag="ps_s")
                            for jj in range(2):
                                jt = jp + jj
                                for ck in range(CT):
                                    nc.tensor.matmul(
                                        ps2[:, jj, 0:cw],
                                        K[:, ck, jt * 128 : (jt + 1) * 128],
                                        Q[:, ck, c0 : c0 + cw],
                                        start=(ck == 0),
                                        stop=(ck == CT - 1),
                                    )
                            nc.scalar.activation(
                                e[:, jp : jp + 2, 0:cw], ps2[:, :, 0:cw], Exp
                            )

                        return act

                    return [mk(jp) for jp in range(0, NT, 2)]

                def out_actions(e, VT, xt_dram, yt_dram, c0, cw):
                    # actions = out-matmul slices + epilogue, per i-subtile
                    acts = []
                    for il in range(cw // 128):
                        it = c0 // 128 + il
                        po = psp.tile([128, C + 2], F32, tag="ps_o")

                        xt_t = sp.tile([128, C], F32, tag="xt")

                        def mk_mm(po, il, it, j0, jn, xt_t):
                            def act():
                                if j0 == 0:
                                    nc.sync.dma_start(
                                        out=xt_t[:, :],
                                        in_=xt_dram[it * 128 : (it + 1) * 128, :],
                                    )
                                for jt in range(j0, jn):
                                    nc.tensor.matmul(
                                        po[:, :],
                                        e[:, jt, il * 128 : (il + 1) * 128],
                                        VT[:, jt, :],
                                        start=(jt == 0),
                                        stop=(jt == NT - 1),
                                    )

                            return act

                        for j0 in range(0, NT, 5):
                            acts.append(mk_mm(po, il, it, j0, min(j0 + 5, NT), xt_t))

                        def mk_epi(po, it, xt_t):
                            def act():
                                r = sp.tile([128, 1], F32, tag="r")
                                nc.vector.reciprocal(r[:, :], po[:, C : C + 1])
                                y = sp.tile([128, C], F32, tag="y")
                                nc.vector.scalar_tensor_tensor(
                                    y[:, :],
                                    po[:, 0:C],
                                    r[:, :],
                                    xt_t[:, :],
                                    op0=mybir.AluOpType.mult,
                                    op1=mybir.AluOpType.add,
                                )
                                nc.sync.dma_start(
                                    out=yt_dram[it * 128 : (it + 1) * 128, :], in_=y[:, :]
                                )

                            return act

                        acts.append(mk_epi(po, it, xt_t))
                    return acts

                def weave(a, b):
                    # emit all of a and b interleaved evenly (a paces, b fills)
                    if not b:
                        for f in a:
                            f()
                        return
                    na, nb = len(a), len(b)
                    j = 0
                    for i, f in enumerate(a):
                        f()
                        while j < nb and j * na <= (i + 1) * nb - 1:
                            b[j]()
                            j += 1
                    while j < nb:
                        b[j]()
                        j += 1

                # software pipeline: expS(k) woven with out(k-1); dir-2 projections
                # are distributed as extra fill across dir-1's chunks (they MUST
                # all be emitted before dir-2's first expS reads Q2/K1/VT1)
                plan = [
                    (proj["Q1"], proj["K2"], proj["VT2"], xt_d["x1t"], y_d["y1t"], c0, cw)
                    for c0, cw in CHUNKS
                ] + [
                    (proj["Q2"], proj["K1"], proj["VT1"], xt_d["x2t"], y_d["y2t"], c0, cw)
                    for c0, cw in CHUNKS
                ]
                nd1 = len(CHUNKS)
                quota = (len(fill) + nd1 - 1) // nd1
                pending = []
                for step, (Q, K, VT, xtd, ytd, c0, cw) in enumerate(plan):
                    if step < nd1:
                        extra, fill = fill[:quota], fill[quota:]
                    else:
                        assert not fill
                        extra = []
                    e = ep.tile([128, NT, CW], BF16, tag="e")
                    weave(exp_actions(Q, K, e, c0, cw), pending + extra)
                    pending = out_actions(e, VT, xtd, ytd, c0, cw)
                weave(pending, [])

    nc.compile()
    return nc


def _get_nc():
    if "nc" not in _CACHE:
        _CACHE["nc"] = _build()
    return _CACHE["nc"]


def kernel(
    x1,
    x2,
    w_q1,
    b_q1,
    w_k1,
    b_k1,
    w_v1,
    b_v1,
    w_q2,
    b_q2,
    w_k2,
    b_k2,
    w_v2,
    b_v2,
    _trace=False,
):
    from concourse.bass_utils import run_bass_kernel_spmd

    nc = _get_nc()

    x1 = np.asarray(x1, dtype=np.float32)
    x2 = np.asarray(x2, dtype=np.float32)
    x1h = x1.astype(np.float16)
    x2h = x2.astype(np.float16)
    # wpack order must match w_names: wq1t, wk2t, wv2t, wq2t, wk1t, wv1t
    wpack = np.ascontiguousarray(
        np.concatenate(
            [np.asarray(w, np.float32).T for w in [w_q1, w_k2, w_v2, w_q2, w_k1, w_v1]],
            axis=1,
        ).astype(np.float16)
    )
    bqk = np.ascontiguousarray(
        np.stack(
            [np.asarray(b, np.float32) for b in [b_q1, b_k1, b_q2, b_k2]], axis=1
        )
    )
    bv1 = np.concatenate(
        [np.asarray(b_v1, np.float32).reshape(1, C), np.ones((1, 2), np.float32)], 1
    )
    bv2 = np.concatenate(
        [np.asarray(b_v2, np.float32).reshape(1, C), np.ones((1, 2), np.float32)], 1
    )
    bvpack = np.concatenate([bv1, bv2, np.ones((1, 128), np.float32)], 1).astype(
        np.float16
    )

    in_maps = []
    for i in range(B):
        x1i = np.ascontiguousarray(x1[i].reshape(C, N))
        x2i = np.ascontiguousarray(x2[i].reshape(C, N))
        m = {
            "x1": np.ascontiguousarray(x1h[i].reshape(C, N)),
            "x2": np.ascontiguousarray(x2h[i].reshape(C, N)),
            "x1t": np.ascontiguousarray(x1i.T),
            "x2t": np.ascontiguousarray(x2i.T),
            "wpack": wpack,
            "bqk": bqk,
            "bvpack": bvpack,
        }
        in_maps.append(m)

    res = run_bass_kernel_spmd(nc, in_maps, list(range(B)), trace=_trace)
    if _trace:
        _CACHE["last_result"] = res

    y1 = np.empty((B, C, H, W), np.float32)
    y2 = np.empty((B, C, H, W), np.float32)
    for i in range(B):
        y1[i] = res.results[i]["y1t"].T.reshape(C, H, W)
        y2[i] = res.results[i]["y2t"].T.reshape(C, H, W)
    return y1, y2


# revision 22
# speedup vs baseline: 1.0161x; 1.0089x over previous
"""Bidirectional cross-attention Trainium2 kernel.

Data-parallel over batch B=8 across 8 NeuronCores (1 sample/core).

Per-core dataflow (matmul operands fp16, attention weights bf16, fp32 accum):
  Q1[c,n], K2[c,n]   : 1x1-conv projections (fp16), per-partition bias on ACT/DVE
  V2T[n,c]+bias (bf16), plus ones columns at c=256/257 (bias folded via K=1 matmul)
  expS1T[j,i] = exp(K2^T Q1)  (ScalarE exp -> bf16; no max-subtract: logits are
                               bounded ~|41| for this problem's scale, exp fits fp32/bf16)
  outT[i, 0:258] = sum_j expS1T[j,i] * [V2T+b | 1 | 1][j, :]
     -> col 256 is the softmax denominator; y = outT[:,0:256]/denominator + x1T
  (symmetric for direction 2)
Chunks of 512 i-columns are software-pipelined: expS(k) matmuls+exps woven with
out(k-1) matmuls so PE never waits on ScalarE; dummy warm-up matmuls keep the
PE HAM clock at 2.4GHz during the input DMA window.

Host side: shard batch over cores, transpose weights/x, gather + transpose outputs.
"""

import sys

if "/opt/trn_rl_repo" not in sys.path:
    sys.path.insert(0, "/opt/trn_rl_repo")

import numpy as np

B, C, H, W = 8, 256, 48, 48
N = H * W  # 2304
NT = N // 128  # 18 j/i tiles
CT = C // 128  # 2 c tiles
CW = 512  # max i-chunk width for expS (last chunk is 256)
CHUNKS = [(0, 512), (512, 512), (1024, 512), (1536, 512), (2048, 256)]

_CACHE = {}


def _build():
    import concourse.bacc as bacc
    import concourse.mybir as mybir
    from concourse.tile import TileContext

    F32, F16, BF16 = mybir.dt.float32, mybir.dt.float16, mybir.dt.bfloat16
    Exp = mybir.ActivationFunctionType.Exp
    Ident = mybir.ActivationFunctionType.Identity

    nc = bacc.Bacc(None, target_bir_lowering=False)

    x_d = {
        "x1": nc.dram_tensor("x1", [C, N], F16, kind="ExternalInput"),
        "x2": nc.dram_tensor("x2", [C, N], F16, kind="ExternalInput"),
    }
    xt_d = {
        "x1t": nc.dram_tensor("x1t", [N, C], F32, kind="ExternalInput"),
        "x2t": nc.dram_tensor("x2t", [N, C], F32, kind="ExternalInput"),
    }
    w_names = ["wq1t", "wk2t", "wv2t", "wq2t", "wk1t", "wv1t"]  # pack order
    wpack_d = nc.dram_tensor("wpack", [C, 6 * C], F16, kind="ExternalInput")
    bqk_names = ["bq1", "bk1", "bq2", "bk2"]  # pack order
    bqk_d = nc.dram_tensor("bqk", [C, 4], F32, kind="ExternalInput")
    # bvpack: [bv1(258) | bv2(258) | ones(128)]
    bvpack_d = nc.dram_tensor("bvpack", [1, 644], F16, kind="ExternalInput")
    y_d = {
        "y1t": nc.dram_tensor("y1t", [N, C], F32, kind="ExternalOutput"),
        "y2t": nc.dram_tensor("y2t", [N, C], F32, kind="ExternalOutput"),
    }

    with TileContext(nc) as tc:
        with (
            tc.tile_pool(name="const", bufs=1) as cp,
            tc.tile_pool(name="proj", bufs=1) as pp,
            tc.tile_pool(name="stream", bufs=4) as sp,
            tc.tile_pool(name="psum", bufs=2, space="PSUM") as psp,
            tc.tile_pool(name="psum_s", bufs=3, space="PSUM") as psp2,
        ):
            # ---------- setup: warm-up + input loads ----------
            proj = {}
            # PE warm-up while input DMAs are in flight: keeps HAM at 8/8
            dummy = cp.tile([128, 512], F16, tag="warm")
            nc.vector.memset(dummy[:, :], 0.0)
            wps = psp.tile([128, 512], F32, tag="ps_o")
            for _ in range(52):
                nc.tensor.matmul(
                    wps[:, :], dummy[:, 0:128], dummy[:, :], start=True, stop=True
                )
            wexp = cp.tile([128, 512], F32, tag="warm_exp")
            nc.scalar.activation(wexp[:, :], wps[:, :], Exp)

            x_sb = {}

            def load_x(n):
                t = pp.tile([128, CT, N], F16, tag=n)
                for ck in range(CT):
                    for h0 in (0, N // 2):
                        nc.sync.dma_start(
                            out=t[:, ck, h0 : h0 + N // 2],
                            in_=x_d[n][ck * 128 : (ck + 1) * 128, h0 : h0 + N // 2],
                        )
                x_sb[n] = t

            load_x("x1")
            wpack = cp.tile([128, CT, 6 * C], F16, tag="wpack")
            for ck in range(CT):
                nc.sync.dma_start(
                    out=wpack[:, ck, :], in_=wpack_d[ck * 128 : (ck + 1) * 128, :]
                )
            w_sb = {n: wpack[:, :, i * C : (i + 1) * C] for i, n in enumerate(w_names)}
            bqkt = cp.tile([128, CT, 4], F32, tag="bqkt")
            for ck in range(CT):
                nc.sync.dma_start(
                    out=bqkt[:, ck, :], in_=bqk_d[ck * 128 : (ck + 1) * 128, :]
                )
            bqk_sb = {n: bqkt[:, :, i] for i, n in enumerate(bqk_names)}
            bvpack = cp.tile([1, 644], F16, tag="bvpack")
            nc.sync.dma_start(out=bvpack[:, :], in_=bvpack_d[:, :])
            bv_sb = {"bv1": bvpack[:, 0:258], "bv2": bvpack[:, 258:516]}
            ones_sb = bvpack[:, 516:644]
            load_x("x2")

            # ---------- projection action builders ----------
            def proj_qk_actions(dst, xt, wn, bn, alt0=0):
                acts = []
                i = 0
                for ct in range(CT):
                    for c0, cw in CHUNKS:

                        def mk(ct, c0, cw, use_act):
                            def act():
                                ps2 = psp2.tile([128, 2, CW], F32, tag="ps_s")
                                ps = ps2[:, 0, :]
                                for ck in range(CT):
                                    nc.tensor.matmul(
                                        ps[:, 0:cw],
                                        w_sb[wn][:, ck, ct * 128 : (ct + 1) * 128],
                                        xt[:, ck, c0 : c0 + cw],
                                        start=(ck == 0),
                                        stop=(ck == CT - 1),
                                    )
                                if use_act:
                                    nc.scalar.activation(
                                        dst[:, ct, c0 : c0 + cw],
                                        ps[:, 0:cw],
                                        Ident,
                                        bias=bqk_sb[bn][:, ct : ct + 1],
                                    )
                                else:
                                    nc.vector.tensor_scalar_add(
                                        dst[:, ct, c0 : c0 + cw],
                                        ps[:, 0:cw],
                                        bqk_sb[bn][:, ct : ct + 1],
                                    )

                            return act

                        acts.append(mk(ct, c0, cw, (alt0 + i) % 2 == 0))
                        i += 1
                return acts

            def proj_vt_actions(dst, xt, wn, bn):
                acts = []
                for jt in range(NT):

                    def mk(jt):
                        def act():
                            ps2 = psp2.tile([128, 2, CW], F32, tag="ps_s")
                            ps = ps2[:, 0, :]
                            for ck in range(CT):
                                nc.tensor.matmul(
                                    ps[:, 0:C],
                                    xt[:, ck, jt * 128 : (jt + 1) * 128],
                                    w_sb[wn][:, ck, :],
                                    start=(ck == 0),
                                    stop=False,
                                    skip_group_check=True,
                                )
                            nc.tensor.matmul(
                                ps[:, 0 : C + 2],
                                ones_sb,
                                bv_sb[bn],
                                start=False,
                                stop=True,
                                skip_group_check=True,
                            )
                            if jt % 2 == 0:
                                nc.vector.tensor_copy(dst[:, jt, :], ps[:, 0 : C + 2])
                            else:
                                nc.scalar.activation(
                                    dst[:, jt, :], ps[:, 0 : C + 2], Ident
                                )

                        return act

                    acts.append(mk(jt))
                return acts

            for nm in ["Q1", "K2", "Q2", "K1"]:
                proj[nm] = pp.tile([128, CT, N], F16, tag=nm, name=nm)
            for nm in ["VT2", "VT1"]:
                proj[nm] = pp.tile([128, NT, C + 2], BF16, tag=nm, name=nm)

            # only Q1/K2 must precede dir-1 attention; VT2 is consumed by
            # out(c0) whose emission starts in chunk 1, so VT2 and all dir-2
            # projections become fill work woven into dir-1's attention chunks
            for a in proj_qk_actions(proj["Q1"], x_sb["x1"], "wq1t", "bq1", 0):
                a()
            for a in proj_qk_actions(proj["K2"], x_sb["x2"], "wk2t", "bk2", 1):
                a()
            vt2_acts = proj_vt_actions(proj["VT2"], x_sb["x2"], "wv2t", "bv2")
            fill = (
                vt2_acts
                + proj_qk_actions(proj["Q2"], x_sb["x2"], "wq2t", "bq2", 0)
                + proj_qk_actions(proj["K1"], x_sb["x1"], "wk1t", "bk1", 1)
                + proj_vt_actions(proj["VT1"], x_sb["x1"], "wv1t", "bv1")
            )
            # per-chunk fill quotas: ALL of VT2 must be emitted within chunk 0
            n_vt2 = len(vt2_acts)
            rest = len(fill) - n_vt2 - 4
            quotas = [n_vt2 + 4] + [(rest + 3) // 4] * 4

            # ---------- attention ----------
            with tc.tile_pool(name="ep", bufs=2) as ep:

                def exp_actions(Q, K, e, c0, cw):
                    # one action = expS matmuls + one wide exp for a PAIR of j-tiles
                    def mk(jp):
                        def act():
                            ps2 = psp2.tile([128, 2, CW], F32, tag="ps_s")
                            for jj in range(2):
                                jt = jp + jj
                                for ck in range(CT):
                                    nc.tensor.matmul(
                                        ps2[:, jj, 0:cw],
                                        K[:, ck, jt * 128 : (jt + 1) * 128],
                                        Q[:, ck, c0 : c0 + cw],
                                        start=(ck == 0),
                                        stop=(ck == CT - 1),
                                    )
                            nc.scalar.activation(
                                e[:, jp : jp + 2, 0:cw], ps2[:, :, 0:cw], Exp
                            )

                        return act

                    return [mk(jp) for jp in range(0, NT, 2)]

                def out_actions(e, VT, xt_dram, yt_dram, c0, cw):
                    # actions = out-matmul slices + epilogue, per i-subtile
                    acts = []
                    for il in range(cw // 128):
                        it = c0 // 128 + il
                        po = psp.tile([128, C + 2], F32, tag="ps_o")

                        xt_t = sp.tile([128, C], F32, tag="xt")

                        def mk_mm(po, il, it, j0, jn, xt_t):
                            def act():
                                if j0 == 0:
                                    nc.sync.dma_start(
                                        out=xt_t[:, :],
                                        in_=xt_dram[it * 128 : (it + 1) * 128, :],
                                    )
                                for jt in range(j0, jn):
                                    nc.tensor.matmul(
                                        po[:, :],
                                        e[:, jt, il * 128 : (il + 1) * 128],
                                        VT[:, jt, :],
                                        start=(jt == 0),
                                        stop=(jt == NT - 1),
                                    )

                            return act

                        for j0 in range(0, NT, 5):
                            acts.append(mk_mm(po, il, it, j0, min(j0 + 5, NT), xt_t))

                        def mk_epi(po, it, xt_t):
                            def act():
                                r = sp.tile([128, 1], F32, tag="r")
                                nc.vector.reciprocal(r[:, :], po[:, C : C + 1])
                                y = sp.tile([128, C], F32, tag="y")
                                nc.vector.scalar_tensor_tensor(
                                    y[:, :],
                                    po[:, 0:C],
                                    r[:, :],
                                    xt_t[:, :],
                                    op0=mybir.AluOpType.mult,
                                    op1=mybir.AluOpType.add,
                                )
                                nc.sync.dma_start(
                                    out=yt_dram[it * 128 : (it + 1) * 128, :], in_=y[:, :]
                                )

                            return act

                        acts.append(mk_epi(po, it, xt_t))
                    return acts

                def weave(a, b):
                    # emit all of a and b interleaved evenly (a paces, b fills)
                    if not b:
                        for f in a:
                            f()
                        return
                    na, nb = len(a), len(b)
                    j = 0
                    for i, f in enumerate(a):
                        f()
                        while j < nb and j * na <= (i + 1) * nb - 1:
                            b[j]()
                            j += 1
                    while j < nb:
                        b[j]()
                        j += 1

                # software pipeline: expS(k) woven with out(k-1); dir-2 projections
                # are distributed as extra fill across dir-1's chunks (they MUST
                # all be emitted before dir-2's first expS reads Q2/K1/VT1)
                plan = [
                    (proj["Q1"], proj["K2"], proj["VT2"], xt_d["x1t"], y_d["y1t"], c0, cw)
                    for c0, cw in CHUNKS
                ] + [
                    (proj["Q2"], proj["K1"], proj["VT1"], xt_d["x2t"], y_d["y2t"], c0, cw)
                    for c0, cw in CHUNKS
                ]
                nd1 = len(CHUNKS)
                pending = []
                for step, (Q, K, VT, xtd, ytd, c0, cw) in enumerate(plan):
                    if step < nd1:
                        q = quotas[step]
                        extra, fill = fill[:q], fill[q:]
                    else:
                        assert not fill
                        extra = []
                    e = ep.tile([128, NT, CW], BF16, tag="e")
                    weave(exp_actions(Q, K, e, c0, cw), pending + extra)
                    pending = out_actions(e, VT, xtd, ytd, c0, cw)
                weave(pending, [])

    nc.compile()
    return nc


def _get_nc():
    if "nc" not in _CACHE:
        _CACHE["nc"] = _build()
    return _CACHE["nc"]


def kernel(
    x1,
    x2,
    w_q1,
    b_q1,
    w_k1,
    b_k1,
    w_v1,
    b_v1,
    w_q2,
    b_q2,
    w_k2,
    b_k2,
    w_v2,
    b_v2,
    _trace=False,
):
    from concourse.bass_utils import run_bass_kernel_spmd

    nc = _get_nc()

    x1 = np.asarray(x1, dtype=np.float32)
    x2 = np.asarray(x2, dtype=np.float32)
    x1h = x1.astype(np.float16)
    x2h = x2.astype(np.float16)
    # wpack order must match w_names: wq1t, wk2t, wv2t, wq2t, wk1t, wv1t
    wpack = np.ascontiguousarray(
        np.concatenate(
            [np.asarray(w, np.float32).T for w in [w_q1, w_k2, w_v2, w_q2, w_k1, w_v1]],
            axis=1,
        ).astype(np.float16)
    )
    bqk = np.ascontiguousarray(
        np.stack(
            [np.asarray(b, np.float32) for b in [b_q1, b_k1, b_q2, b_k2]], axis=1
        )
    )
    bv1 = np.concatenate(
        [np.asarray(b_v1, np.float32).reshape(1, C), np.ones((1, 2), np.float32)], 1
    )
    bv2 = np.concatenate(
        [np.asarray(b_v2, np.float32).reshape(1, C), np.ones((1, 2), np.float32)], 1
    )
    bvpack = np.concatenate([bv1, bv2, np.ones((1, 128), np.float32)], 1).astype(
        np.float16
    )

    in_maps = []
    for i in range(B):
        x1i = np.ascontiguousarray(x1[i].reshape(C, N))
        x2i = np.ascontiguousarray(x2[i].reshape(C, N))
        m = {
            "x1": np.ascontiguousarray(x1h[i].reshape(C, N)),
            "x2": np.ascontiguousarray(x2h[i].reshape(C, N)),
            "x1t": np.ascontiguousarray(x1i.T),
            "x2t": np.ascontiguousarray(x2i.T),
            "wpack": wpack,
            "bqk": bqk,
            "bvpack": bvpack,
        }
        in_maps.append(m)

    res = run_bass_kernel_spmd(nc, in_maps, list(range(B)), trace=_trace)
    if _trace:
        _CACHE["last_result"] = res

    y1 = np.empty((B, C, H, W), np.float32)
    y2 = np.empty((B, C, H, W), np.float32)
    for i in range(B):
        y1[i] = res.results[i]["y1t"].T.reshape(C, H, W)
        y2[i] = res.results[i]["y2t"].T.reshape(C, H, W)
    return y1, y2
